# revision 1
# baseline (speedup 1.0000x reference)
"""AttentiveItemToVec Trainium2 kernel.

Full-input contract: kernel(**inputs) takes the unsharded numpy inputs and
returns the full [512, 101, 128] float32 output. Internally shards the batch
across 8 NeuronCores (64 batches each), runs a Bass/Tile kernel per core via
run_bass_kernel_spmd, and concatenates the per-core outputs.

Per-core (64 batches): embedding rows are fetched with multi-block indirect
DMAs (4x128 rows per instruction, padded index layout so batch b's rows land
on partitions 0..100 of block b). Per batch: PE-transpose v/u to
feature-major, project (tpT/cpT with bias), squared norms via
matmul-with-ones, 1/|x| = exp(-0.5*ln(x^2)) on ScalarE (Ln/Exp/Copy/Identity
are forced into one activation table, so the kernel pays a single table
load), cosine scores, softmax without max-subtraction (cos is in [-1,1];
pad-mask enters the exp as a -1e30 bias), attention apply, output
projection. Softmax normalization and the Bc_b/R_b biases are folded into
the output stage (attention rows sum to 1).
"""

import numpy as np
from contextlib import ExitStack

# Problem constants (hardcoded per contract).
V, E, D = 100000, 128, 60
B, J, M, P = 512, 101, 50, 5120
NCORES = 8
BLOC = B // NCORES  # 64 batches per core
NEG = -1.0e30
EPS2 = 1e-12  # clamp on squared norms (eps=1e-6 on norms)

_CACHE = {}

_ACT_TABLE = "natural_log_exp_and_others"


def _patched_tables(orig_fn):
    def fn(arch):
        tabs = orig_fn(arch)
        return {
            name: (s if name == _ACT_TABLE else type(s)())
            for name, s in tabs.items()
        }
    return fn


def _build_program():
    import os
    NOPATCH = os.environ.get("K_NOPATCH") == "1"
    import concourse.bass as bass
    import concourse.tile as tile
    import concourse.bacc as bacc_mod
    from concourse import bacc, mybir

    f32 = mybir.dt.float32
    i32 = mybir.dt.int32

    nc = bacc.Bacc(
        "TRN2",
        target_bir_lowering=False,
        debug=False,
        enable_asserts=False,
    )

    temb = nc.dram_tensor("t_emb", [V, E], f32, kind="ExternalInput").ap()
    cemb = nc.dram_tensor("c_emb", [V, E], f32, kind="ExternalInput").ap()
    atwT = nc.dram_tensor("atwT", [E, D], f32, kind="ExternalInput").ap()
    acwT = nc.dram_tensor("acwT", [E, D], f32, kind="ExternalInput").ap()
    bcwT = nc.dram_tensor("bcwT", [E, E], f32, kind="ExternalInput").ap()
    rwT = nc.dram_tensor("rwT", [E, E], f32, kind="ExternalInput").ap()
    atb = nc.dram_tensor("atb", [D, 1], f32, kind="ExternalInput").ap()
    acb = nc.dram_tensor("acb", [D, 1], f32, kind="ExternalInput").ap()
    rbeff = nc.dram_tensor("rbeff", [1, E], f32, kind="ExternalInput").ap()
    eye = nc.dram_tensor("eye", [128, 128], f32, kind="ExternalInput").ap()
    eyehi = nc.dram_tensor("eyehi", [128, 64], f32, kind="ExternalInput").ap()
    offt = nc.dram_tensor("offt", [128, BLOC], i32, kind="ExternalInput").ap()
    offc = nc.dram_tensor("offc", [128, BLOC // 2], i32, kind="ExternalInput").ap()
    maskT = nc.dram_tensor("maskT", [M, BLOC], f32, kind="ExternalInput").ap()
    out = nc.dram_tensor("out", [BLOC, J, E], f32, kind="ExternalOutput").ap()

    AF = mybir.ActivationFunctionType

    with tile.TileContext(nc) as tc, ExitStack() as ctx:
        const = ctx.enter_context(tc.tile_pool(name="const", bufs=1))
        vgp = ctx.enter_context(tc.tile_pool(name="vg", bufs=BLOC // 4))
        ugp = ctx.enter_context(tc.tile_pool(name="ug", bufs=BLOC // 8))
        work = ctx.enter_context(tc.tile_pool(name="work", bufs=5))
        vecp = ctx.enter_context(tc.tile_pool(name="vec", bufs=6))
        outp = ctx.enter_context(tc.tile_pool(name="outp", bufs=3))
        psb = ctx.enter_context(tc.tile_pool(name="psb", bufs=3, space="PSUM"))
        psd = ctx.enter_context(tc.tile_pool(name="psd", bufs=3, space="PSUM"))
        psv = ctx.enter_context(tc.tile_pool(name="psv", bufs=2, space="PSUM"))

        # --- constants ---
        eye_t = const.tile([128, 128], f32)
        nc.sync.dma_start(out=eye_t[:], in_=eye[:, :])
        eyehi_t = const.tile([128, 64], f32)
        nc.sync.dma_start(out=eyehi_t[:], in_=eyehi[:, :])
        atwT_t = const.tile([E, D], f32)
        nc.sync.dma_start(out=atwT_t[:], in_=atwT[:, :])
        acwT_t = const.tile([E, D], f32)
        nc.sync.dma_start(out=acwT_t[:], in_=acwT[:, :])
        bcwT_t = const.tile([E, E], f32)
        nc.sync.dma_start(out=bcwT_t[:], in_=bcwT[:, :])
        rwT_t = const.tile([E, E], f32)
        nc.sync.dma_start(out=rwT_t[:], in_=rwT[:, :])
        atb_t = const.tile([D, 1], f32)
        nc.sync.dma_start(out=atb_t[:], in_=atb[:, :])
        acb_t = const.tile([D, 1], f32)
        nc.sync.dma_start(out=acb_t[:], in_=acb[:, :])
        rb_t = const.tile([128, E], f32)
        rb_bcast = bass.AP(tensor=rbeff.tensor, offset=0, ap=[[0, 128], [1, E]])
        nc.sync.dma_start(out=rb_t[:], in_=rb_bcast)
        offt_t = const.tile([128, BLOC], i32)
        nc.sync.dma_start(out=offt_t[:], in_=offt[:, :])
        offc_t = const.tile([128, BLOC // 2], i32)
        nc.sync.dma_start(out=offc_t[:], in_=offc[:, :])
        maskT_t = const.tile([M, BLOC], f32)
        nc.sync.dma_start(out=maskT_t[:], in_=maskT[:, :])
        ones_t = const.tile([128, 1], f32)
        nc.vector.memset(ones_t[:], 1.0)
        eps_t = const.tile([128, 1], f32)
        nc.vector.memset(eps_t[:], EPS2)

        # --- gathers: 4 blocks of 128 rows per indirect DMA instruction.
        # batch b's 101 target rows = partitions 0..100 of v block b;
        # batch b's 50 context rows = partitions 64*(b%2).. of u block b//2
        vg = [None] * (BLOC // 4)
        ug = [None] * (BLOC // 8)
        for t in range(BLOC // 8):
            for qq in (2 * t, 2 * t + 1):
                g = vgp.tile([128, 4, E], f32, tag="vg")
                for j in range(4):
                    nc.gpsimd.indirect_dma_start(
                        out=g[:, j, :],
                        out_offset=None,
                        in_=temb[:, :],
                        in_offset=bass.IndirectOffsetOnAxis(
                            ap=offt_t[:, 4 * qq + j : 4 * qq + j + 1], axis=0
                        ),
                    )
                vg[qq] = g
            g = ugp.tile([128, 4, E], f32, tag="ug")
            for j in range(4):
                nc.gpsimd.indirect_dma_start(
                    out=g[:, j, :],
                    out_offset=None,
                    in_=cemb[:, :],
                    in_offset=bass.IndirectOffsetOnAxis(
                        ap=offc_t[:, 4 * t + j : 4 * t + j + 1], axis=0
                    ),
                )
            ug[t] = g

        # --- per-batch compute, stage1/stage2 pipelined emission ---
        def stage1(b):
            v_ap = vg[b // 4][:J, b % 4, :]  # [101,128]
            ublk = b // 2
            uo = 64 * (b % 2)
            u_ap = ug[ublk // 4][uo : uo + M, ublk % 4, :]  # [50,128]

            # transposes to feature-major
            vT_ps = psb.tile([128, 128], f32, tag="pbig", space="PSUM")
            nc.tensor.transpose(out=vT_ps[:, :J], in_=v_ap, identity=eye_t[:J, :J])
            vT = work.tile([E, J], f32, tag="vT")
            nc.vector.tensor_copy(out=vT[:], in_=vT_ps[:, :J])

            uT_ps = psb.tile([128, 128], f32, tag="pbig", space="PSUM")
            u_ident = eye_t[:M, :M] if uo == 0 else eyehi_t[uo : uo + M, :M]
            nc.tensor.transpose(out=uT_ps[:, :M], in_=u_ap, identity=u_ident)
            uT = work.tile([E, M], f32, tag="uT")
            nc.scalar.copy(out=uT[:], in_=uT_ps[:, :M])

            # projections (feature-major), bias added during PSUM->SBUF copy
            tpT_ps = psb.tile([128, 128], f32, tag="pbig", space="PSUM")
            nc.tensor.matmul(
                out=tpT_ps[:D, :J], lhsT=atwT_t[:], rhs=vT[:], start=True, stop=True
            )
            tpT = work.tile([D, J], f32, tag="tpT")
            nc.scalar.activation(
                out=tpT[:], in_=tpT_ps[:D, :J], func=AF.Identity, bias=atb_t[:], scale=1.0
            )

            cpT_ps = psb.tile([128, 128], f32, tag="pbig", space="PSUM")
            nc.tensor.matmul(
                out=cpT_ps[:D, :M], lhsT=acwT_t[:], rhs=uT[:], start=True, stop=True
            )
            cpT = work.tile([D, M], f32, tag="cpT")
            nc.scalar.activation(
                out=cpT[:], in_=cpT_ps[:D, :M], func=AF.Identity, bias=acb_t[:], scale=1.0
            )

            # squared norms via matmul-with-ones -> column vectors
            tpT2 = work.tile([D, J], f32, tag="tpT2")
            nc.vector.tensor_mul(out=tpT2[:], in0=tpT[:], in1=tpT[:])
            cpT2 = work.tile([D, M], f32, tag="cpT2")
            nc.vector.tensor_mul(out=cpT2[:], in0=cpT[:], in1=cpT[:])

            nt2_ps = psv.tile([128, 1], f32, tag="pvec", space="PSUM")
            nc.tensor.matmul(
                out=nt2_ps[:J, :], lhsT=tpT2[:], rhs=ones_t[:D, :], start=True, stop=True
            )
            nc2_ps = psv.tile([128, 1], f32, tag="pvec", space="PSUM")
            nc.tensor.matmul(
                out=nc2_ps[:M, :], lhsT=cpT2[:], rhs=ones_t[:D, :], start=True, stop=True
            )

            # 1/|x| = exp(-0.5 * ln(x^2 + eps)) -- Ln and Exp share one table
            lnt = vecp.tile([128, 1], f32, tag="lnt")
            nc.scalar.activation(
                out=lnt[:J], in_=nt2_ps[:J, :], func=AF.Ln, bias=eps_t[:J, :]
            )
            ntinv = vecp.tile([128, 1], f32, tag="ntinv")
            nc.scalar.activation(
                out=ntinv[:J], in_=lnt[:J], func=AF.Exp, scale=-0.5
            )

            lnc = vecp.tile([128, 1], f32, tag="lnc")
            nc.scalar.activation(
                out=lnc[:M], in_=nc2_ps[:M, :], func=AF.Ln, bias=eps_t[:M, :]
            )
            ncinv = vecp.tile([128, 1], f32, tag="ncinv")
            nc.scalar.activation(
                out=ncinv[:M], in_=lnc[:M], func=AF.Exp, scale=-0.5
            )

            # dot products (own double-buffered bank; spans into stage2)
            dot_ps = psd.tile([128, 128], f32, tag="pdot", space="PSUM")
            nc.tensor.matmul(
                out=dot_ps[:J, :M], lhsT=tpT[:], rhs=cpT[:], start=True, stop=True
            )
            return dot_ps, ntinv, ncinv, uT

        def stage2(b, st):
            dot_ps, ntinv, ncinv, uT = st
            dotn = work.tile([J, M], f32, tag="dotn")
            nc.vector.tensor_scalar_mul(dotn[:], dot_ps[:J, :M], ntinv[:J, :])

            # transpose to [50,101]; exp(ncinv*x + mask) in one activation
            dotT_ps = psb.tile([128, 128], f32, tag="pbig", space="PSUM")
            nc.tensor.transpose(
                out=dotT_ps[:M, :J], in_=dotn[:], identity=eye_t[:J, :J]
            )
            attnT = work.tile([M, J], f32, tag="attnT")
            nc.scalar.activation(
                out=attnT[:],
                in_=dotT_ps[:M, :J],
                func=AF.Exp,
                bias=maskT_t[:, b : b + 1],
                scale=ncinv[:M, :],
            )

            # softmax denominators (per target row j)
            cs_ps = psv.tile([128, 1], f32, tag="pvec", space="PSUM")
            nc.tensor.matmul(
                out=cs_ps[:J, :], lhsT=attnT[:], rhs=ones_t[:M, :], start=True, stop=True
            )
            rsinv = vecp.tile([128, 1], f32, tag="rsinv")
            nc.vector.reciprocal(out=rsinv[:J], in_=cs_ps[:J, :1])


            # bu = u @ Bc_w.T (bias folded into rbeff), then alphaT, then output

            bu_ps = psb.tile([128, 128], f32, tag="pbig", space="PSUM")
            nc.tensor.matmul(
                out=bu_ps[:M, :E], lhsT=uT[:], rhs=bcwT_t[:], start=True, stop=True
            )
            bu = work.tile([M, E], f32, tag="bu")
            nc.scalar.copy(out=bu[:], in_=bu_ps[:M, :])

            al_ps = psb.tile([128, 128], f32, tag="pbig", space="PSUM")
            nc.tensor.matmul(
                out=al_ps[:E, :J], lhsT=bu[:], rhs=attnT[:], start=True, stop=True
            )
            alT = work.tile([E, J], f32, tag="alT")
            nc.vector.tensor_copy(out=alT[:], in_=al_ps[:, :J])

            o_ps = psb.tile([128, 128], f32, tag="pbig", space="PSUM")
            nc.tensor.matmul(
                out=o_ps[:J, :E], lhsT=alT[:], rhs=rwT_t[:], start=True, stop=True
            )
            o_sb = outp.tile([J, E], f32, tag="o")
            nc.vector.scalar_tensor_tensor(
                out=o_sb[:], in0=o_ps[:J, :E], scalar=rsinv[:J, :],
                in1=rb_t[:J, :], op0=mybir.AluOpType.mult,
                op1=mybir.AluOpType.add,
            )
            nc.sync.dma_start(out=out[b], in_=o_sb[:])

        # per-engine execution is in program order: interleave batch b+1's
        # stage1 with batch b's stage2 so independent work hides the waits
        LAG = 2
        pend = [(0, stage1(0))]
        for b in range(1, BLOC):
            pend.append((b, stage1(b)))
            if len(pend) > LAG:
                stage2(*pend.pop(0))
        while pend:
            stage2(*pend.pop(0))

    # Force every activation onto the one table holding Ln/Exp/Copy/Identity
    # so the kernel pays a single table load. Indices into act_info.json are
    # preserved (other sets are just emptied for the placement pass), so the
    # runtime table mapping stays correct.
    if NOPATCH:
        nc.compile()
    else:
        orig = bacc_mod.get_activation_tables
        bacc_mod.get_activation_tables = _patched_tables(orig)
        try:
            nc.compile()
        finally:
            bacc_mod.get_activation_tables = orig
    return nc


def _get_program():
    if "nc" not in _CACHE:
        _CACHE["nc"] = _build_program()
    return _CACHE["nc"]


def _prep_inputs(batch_titems, batch_citems, batch_pad_ids, t_emb, c_emb,
                 Ac_w, Ac_b, At_w, At_b, Bc_w, Bc_b, R_w, R_b):
    f = lambda x: np.ascontiguousarray(np.asarray(x, dtype=np.float32))
    t_emb = f(t_emb)
    c_emb = f(c_emb)
    tit = np.asarray(batch_titems).astype(np.int32)
    cit = np.asarray(batch_citems).astype(np.int32)
    pad = np.asarray(batch_pad_ids).astype(np.int64)

    mask = np.zeros((B, M), np.float32)
    mask[pad[0], pad[1]] = NEG

    atwT = f(np.asarray(At_w).T)
    acwT = f(np.asarray(Ac_w).T)
    bcwT = f(np.asarray(Bc_w).T)
    rwT = f(np.asarray(R_w).T)
    atb = f(np.asarray(At_b).reshape(D, 1))
    acb = f(np.asarray(Ac_b).reshape(D, 1))
    rbeff = f(
        (np.asarray(R_b, np.float32)
         + np.asarray(R_w, np.float32) @ np.asarray(Bc_b, np.float32)).reshape(1, E)
    )
    eye = np.eye(128, dtype=np.float32)
    eyehi = np.zeros((128, 64), np.float32)
    eyehi[64:, :] = np.eye(64, dtype=np.float32)

    in_maps = []
    for c in range(NCORES):
        s = c * BLOC
        tslice = tit[s : s + BLOC]  # [64,101]
        tpad = np.zeros((BLOC, 128), np.int32)
        tpad[:, :J] = tslice
        offt = np.ascontiguousarray(tpad.reshape(-1).reshape(BLOC, 128).T)

        cslice = cit[s : s + BLOC]  # [64,50]
        cpad = np.zeros((BLOC, 64), np.int32)
        cpad[:, :M] = cslice
        offc = np.ascontiguousarray(cpad.reshape(-1).reshape(BLOC // 2, 128).T)

        maskTc = np.ascontiguousarray(mask[s : s + BLOC].T)  # [50,64]

        in_maps.append(
            {
                "t_emb": t_emb,
                "c_emb": c_emb,
                "atwT": atwT,
                "acwT": acwT,
                "bcwT": bcwT,
                "rwT": rwT,
                "atb": atb,
                "acb": acb,
                "rbeff": rbeff,
                "eye": eye,
                "eyehi": eyehi,
                "offt": offt,
                "offc": offc,
                "maskT": maskTc,
            }
        )
    return in_maps


def run_sharded(in_maps, **kwargs):
    from concourse.bass_utils import run_bass_kernel_spmd

    nc = _get_program()
    res = run_bass_kernel_spmd(nc, in_maps, core_ids=list(range(NCORES)), **kwargs)
    outs = [res.results[c]["out"] for c in range(NCORES)]
    full = np.concatenate(outs, axis=0)
    return full, res


def kernel(**inputs):
    in_maps = _prep_inputs(**inputs)
    full, _ = run_sharded(in_maps)
    return full.astype(np.float32)



# revision 2
# speedup vs baseline: 1.0045x; 1.0045x over previous
"""AttentiveItemToVec Trainium2 kernel, v2.

Full-input contract: kernel(**inputs) takes the unsharded numpy inputs and
returns the full [512, 101, 128] float32 output. Internally shards the batch
across 8 NeuronCores (64 batches each), runs a Bass/Tile kernel per core via
run_bass_kernel_spmd, and concatenates the per-core outputs.

Per-core strategy:
- Host dedups this core's token ids (np.unique) into compact bf16 embedding
  tables (int16-indexable) + inverse indices. The device gathers rows with
  gpsimd.dma_gather(transpose=True), which lands them FEATURE-MAJOR in SBUF
  (vT [128, cols]) -- no PE transposes or PSUM evictions for the gathers.
- All matmuls run in bf16 (1 PE cycle/col vs fp32's 4). Projections are
  column-batched across batches (tpT [60, 404] groups etc.) so PSUM->SBUF
  evictions amortize their fixed latency.
- Norms: squares on DVE (bf16 2x), column sums via 1-col ones-matmuls into a
  shared PSUM accumulator, 1/|x| = exp(-0.5*ln(x^2+eps)) batched on Act.
- Attention per batch: dot [101,50] -> ntinv multiply via 0-stride-broadcast
  tensor_tensor (4 batches per DVE op) -> pair-packed transposes (PSUM out
  partition bases 0/64) -> one Exp per batch-pair with per-partition ncinv
  scale + pad-mask bias -> softmax sums via 1-col matmuls -> o matmul ->
  rsinv multiply via 0-stride broadcast (4 batches per DVE op).
- Bc_b/R_b are folded into rbeff = R_b + R_w@Bc_b (attention rows sum to 1);
  when rbeff is all-zero (as in setup_inputs) the bias add is skipped.
"""

import numpy as np
import ml_dtypes
from contextlib import ExitStack

# Problem constants (hardcoded per contract).
V, E, D = 100000, 128, 60
B, J, M, P = 512, 101, 50, 5120
NCORES = 8
BLOC = B // NCORES          # 64 batches per core
NCHUNK = 4                  # gather/compute chunks per core
CB = BLOC // NCHUNK         # 16 batches per chunk
TN = 6528                   # compact t-table rows (padded)
CN = 3200                   # compact c-table rows (padded)
NIV = 896                   # v-gather idxs per HALF-chunk (8*101=808 + 88 pad)
NIC = 896                   # u-gather idxs per chunk (16*50=800 + 96 pad)
NEG = -1.0e30
EPS2 = 1e-12

_CACHE = {}

_ACT_TABLE = "natural_log_exp_and_others"


def _patched_tables(orig_fn):
    def fn(arch):
        tabs = orig_fn(arch)
        return {
            name: (s if name == _ACT_TABLE else type(s)())
            for name, s in tabs.items()
        }
    return fn


def _interleave(*gens):
    gens = [g for g in gens]
    while gens:
        done = []
        for g in gens:
            try:
                next(g)
            except StopIteration:
                done.append(g)
        for g in done:
            gens.remove(g)


def _build_program(has_rb):
    import concourse.bass as bass
    import concourse.tile as tile
    import concourse.bacc as bacc_mod
    from concourse import bacc, mybir

    f32 = mybir.dt.float32
    bf16 = mybir.dt.bfloat16
    i16 = mybir.dt.int16

    nc = bacc.Bacc(
        "TRN2",
        target_bir_lowering=False,
        debug=False,
        enable_asserts=False,
    )

    tcomp = nc.dram_tensor("tcomp", [TN, E], bf16, kind="ExternalInput").ap()
    ccomp = nc.dram_tensor("ccomp", [CN, E], bf16, kind="ExternalInput").ap()
    gidx = nc.dram_tensor(
        "gidx", [128, NCHUNK * (2 * NIV + NIC) // 16], i16, kind="ExternalInput"
    ).ap()
    atwT = nc.dram_tensor("atwT", [E, D], bf16, kind="ExternalInput").ap()
    acwT = nc.dram_tensor("acwT", [E, D], bf16, kind="ExternalInput").ap()
    w2T = nc.dram_tensor("w2T", [E, E], bf16, kind="ExternalInput").ap()
    eyeb = nc.dram_tensor("eyeb", [128, 128], bf16, kind="ExternalInput").ap()
    atb = nc.dram_tensor("atb", [D, 1], f32, kind="ExternalInput").ap()
    acb = nc.dram_tensor("acb", [D, 1], f32, kind="ExternalInput").ap()
    maskP = nc.dram_tensor("maskP", [128, BLOC // 2], f32, kind="ExternalInput").ap()
    onesEO = nc.dram_tensor("onesEO", [128, 2], bf16, kind="ExternalInput").ap()
    rbeff = nc.dram_tensor("rbeff", [1, E], f32, kind="ExternalInput").ap()
    out = nc.dram_tensor("out", [J, BLOC, E], f32, kind="ExternalOutput").ap()

    AF = mybir.ActivationFunctionType
    MUL = mybir.AluOpType.mult
    ADD = mybir.AluOpType.add

    with tile.TileContext(nc) as tc, ExitStack() as ctx:
        const = ctx.enter_context(tc.tile_pool(name="const", bufs=1))
        big = ctx.enter_context(tc.tile_pool(name="big", bufs=1))
        trans = ctx.enter_context(tc.tile_pool(name="trans", bufs=6))
        # PSUM: 8 banks total
        ps_proj = ctx.enter_context(tc.tile_pool(name="psproj", bufs=2, space="PSUM"))
        ps_small = ctx.enter_context(tc.tile_pool(name="pssmall", bufs=1, space="PSUM"))
        ps_dot = ctx.enter_context(tc.tile_pool(name="psdot", bufs=1, space="PSUM"))
        ps_dT = ctx.enter_context(tc.tile_pool(name="psdT", bufs=2, space="PSUM"))
        ps_o = ctx.enter_context(tc.tile_pool(name="pso", bufs=2, space="PSUM"))

        # ---- constants ----
        gidx_t = const.tile([128, NCHUNK * (2 * NIV + NIC) // 16], i16)
        nc.sync.dma_start(out=gidx_t[:], in_=gidx[:, :])
        atwT_t = const.tile([E, D], bf16)
        nc.sync.dma_start(out=atwT_t[:], in_=atwT[:, :])
        acwT_t = const.tile([E, D], bf16)
        nc.sync.dma_start(out=acwT_t[:], in_=acwT[:, :])
        w2T_t = const.tile([E, E], bf16)
        nc.sync.dma_start(out=w2T_t[:], in_=w2T[:, :])
        eye_t = const.tile([128, 128], bf16)
        nc.sync.dma_start(out=eye_t[:], in_=eyeb[:, :])
        atb_t = const.tile([D, 1], f32)
        nc.sync.dma_start(out=atb_t[:], in_=atb[:, :])
        acb_t = const.tile([D, 1], f32)
        nc.sync.dma_start(out=acb_t[:], in_=acb[:, :])
        maskP_t = const.tile([128, BLOC // 2], f32)
        nc.sync.dma_start(out=maskP_t[:], in_=maskP[:, :])
        onesEO_t = const.tile([128, 2], bf16)
        nc.sync.dma_start(out=onesEO_t[:], in_=onesEO[:, :])
        if has_rb:
            rb_t = const.tile([128, E], f32)
            rb_bcast = bass.AP(tensor=rbeff.tensor, offset=0, ap=[[0, 128], [1, E]])
            nc.sync.dma_start(out=rb_t[:], in_=rb_bcast)
        ones_t = const.tile([128, 1], bf16)
        nc.vector.memset(ones_t[:], 1.0)
        eps_t = const.tile([128, 1], f32)
        nc.vector.memset(eps_t[:], EPS2)

        # ---- persistent SBUF tensors ----
        vT = [big.tile([128, 2 * NIV], bf16, tag=f"vT{c}", name=f"vT{c}")
              for c in range(NCHUNK)]
        uT = [big.tile([128, NIC], bf16, tag=f"uT{c}", name=f"uT{c}")
              for c in range(NCHUNK)]
        tpT = big.tile([D, BLOC * J], bf16, tag="tpT")
        CPW = CB * M + 64              # per-chunk cpT stride (incl. junk pad)
        cpT = big.tile([D, NCHUNK * CPW], bf16, tag="cpT")
        buR = big.tile([128, BLOC // 2, 2, E], bf16, tag="buR")
        attnT = big.tile([128, BLOC // 2, J], bf16, tag="attnT")
        ntinv = big.tile([128, BLOC], f32, tag="ntinv")
        ncinvP = big.tile([128, BLOC // 2], f32, tag="ncinvP")
        rsinv = big.tile([128, BLOC], f32, tag="rsinv")
        o_all = big.tile([J, BLOC, E], f32, tag="o_all")

        # shared small PSUM accumulator: nt2 cols 0..63, nc2 cols 64..127
        # (evens at 64+b//2, odds at 96+b//2), cs cols 128..191
        small_ps = ps_small.tile([128, 224], f32, tag="small", space="PSUM")
        # dead partitions 50..63 / 114..127 of ncinvP stay 0 => exp scale is
        # finite there and the pad-mask bias (-1e30) zeroes those attnT rows
        nc.vector.memset(ncinvP[:], 0.0)
        # zero the double-slot buR once; evictions only write the live
        # 50-row parity slices, so the dead rows stay zero and K=114
        # contractions with base-0 operands select one parity per matmul
        nc.vector.memset(buR[:].rearrange("p a b e -> p (a b e)").bitcast(mybir.dt.uint32), 0)

        # ---- gathers (interleaved v/u; num_idxs per instr capped at 896,
        # larger counts crash the gather ucode) ----
        ivc = NIV // 16
        icc = NIC // 16
        for c in range(NCHUNK):
            off = c * (2 * ivc + icc)
            for h in range(2):
                nc.gpsimd.dma_gather(
                    vT[c][:, h * NIV : (h + 1) * NIV].unsqueeze(1), tcomp[:, :],
                    gidx_t[:, off + h * ivc : off + (h + 1) * ivc],
                    NIV, NIV, E, transpose=True,
                )
            nc.gpsimd.dma_gather(
                uT[c][:].unsqueeze(1), ccomp[:, :],
                gidx_t[:, off + 2 * ivc : off + 2 * ivc + icc], NIC, NIC, E,
                transpose=True,
            )

        # ---- PE warmup during the gathers: keep the tensor engine busy so
        # the p-state ramp reaches full clock before real work arrives ----
        for i in range(48):
            wps = ps_dT.tile([128, 8, J + 1], bf16, tag="dT", space="PSUM")
            nc.tensor.transpose(
                out=wps[:, i % 8, :J], in_=eye_t[:J, :], identity=eye_t[:J, :J]
            )

        # ---- per-chunk emission ----
        def proj(c):
            vTc, uTc = vT[c], uT[c]
            # tpT groups: 4 batches (404 cols) each
            for g in range(4):
                b0 = c * CB + 4 * g
                pp = ps_proj.tile([128, 512], f32, tag="proj", space="PSUM")
                vcol = (g // 2) * NIV + (g % 2) * 4 * J
                nc.tensor.matmul(
                    out=pp[:D, : 4 * J], lhsT=atwT_t[:],
                    rhs=vTc[:, vcol : vcol + 4 * J],
                    start=True, stop=True,
                )
                nc.scalar.activation(
                    out=tpT[:, b0 * J : (b0 + 4) * J], in_=pp[:D, : 4 * J],
                    func=AF.Identity, bias=atb_t[:D], scale=1.0,
                )
                sq = trans.tile([D, 4 * J], bf16, tag="sq")
                nc.vector.tensor_tensor(
                    out=sq[:], in0=tpT[:, b0 * J : (b0 + 4) * J],
                    in1=tpT[:, b0 * J : (b0 + 4) * J], op=MUL,
                )
                for k in range(4):
                    nc.tensor.matmul(
                        out=small_ps[:J, b0 + k : b0 + k + 1],
                        lhsT=sq[:, k * J : (k + 1) * J], rhs=ones_t[:D],
                        start=True, stop=True,
                    )
                yield
            # ntinv for the chunk
            lnt = trans.tile([J, CB], f32, tag="lnt")
            nc.scalar.activation(
                out=lnt[:], in_=small_ps[:J, c * CB : (c + 1) * CB],
                func=AF.Ln, bias=eps_t[:J],
            )
            nc.scalar.activation(
                out=ntinv[:J, c * CB : (c + 1) * CB], in_=lnt[:],
                func=AF.Exp, scale=-0.5,
            )
            yield
            # cpT groups: 8 batches (400 cols) each
            for g in range(2):
                b0 = c * CB + 8 * g
                w = 8 * M + (64 if g == 1 else 0)
                cc0 = c * CPW + 8 * g * M
                pp = ps_proj.tile([128, 512], f32, tag="proj", space="PSUM")
                nc.tensor.matmul(
                    out=pp[:D, :w], lhsT=acwT_t[:],
                    rhs=uTc[:, 8 * g * M : 8 * g * M + w],
                    start=True, stop=True,
                )
                nc.vector.tensor_scalar_add(
                    out=cpT[:, cc0 : cc0 + w], in0=pp[:D, :w],
                    scalar1=acb_t[:D],
                )
                sq = trans.tile([D, 8 * M], bf16, tag="sqc")
                nc.vector.tensor_tensor(
                    out=sq[:], in0=cpT[:, cc0 : cc0 + 8 * M],
                    in1=cpT[:, cc0 : cc0 + 8 * M], op=MUL,
                )
                for k in range(8):
                    b = b0 + k
                    col = 64 + 32 * (b % 2) + b // 2
                    nc.tensor.matmul(
                        out=small_ps[0:M, col : col + 1],
                        lhsT=sq[:, k * M : (k + 1) * M], rhs=ones_t[:D],
                        start=True, stop=True,
                    )
                yield
            # ncinvP for the chunk (pair-packed via two parity writes)
            p0 = c * CB // 2
            nP = CB // 2
            lnce = trans.tile([M, nP], f32, tag="lnce")
            nc.scalar.activation(
                out=lnce[:], in_=small_ps[0:M, 64 + p0 : 64 + p0 + nP],
                func=AF.Ln, bias=eps_t[:M],
            )
            nc.scalar.activation(
                out=ncinvP[0:M, p0 : p0 + nP], in_=lnce[:], func=AF.Exp,
                scale=-0.5,
            )
            lnco = trans.tile([M, nP], f32, tag="lnco")
            nc.scalar.activation(
                out=lnco[:], in_=small_ps[0:M, 96 + p0 : 96 + p0 + nP],
                func=AF.Ln, bias=eps_t[:M],
            )
            nc.scalar.activation(
                out=ncinvP[64 : 64 + M, p0 : p0 + nP], in_=lnco[:], func=AF.Exp,
                scale=-0.5,
            )
            yield
            # buR blocks: 2 batches per block, 4 blocks per PSUM bank
            for g in range(2):
                blk0 = c * CB // 2 + 4 * g
                pp = ps_proj.tile([128, 512], f32, tag="proj", space="PSUM")
                ppv = pp[:].rearrange("p (a b) -> p a b", a=4)
                for k in range(4):
                    nc.tensor.matmul(
                        out=ppv[0:M, k, :],
                        lhsT=uTc[:, (8 * g + 2 * k) * M : (8 * g + 2 * k + 1) * M],
                        rhs=w2T_t[:], start=True, stop=True,
                    )
                    nc.tensor.matmul(
                        out=ppv[64 : 64 + M, k, :],
                        lhsT=uTc[:, (8 * g + 2 * k + 1) * M : (8 * g + 2 * k + 2) * M],
                        rhs=w2T_t[:], start=True, stop=True,
                    )
                nc.scalar.copy(
                    out=buR[0:M, blk0 : blk0 + 4, 0, :], in_=ppv[0:M]
                )
                nc.scalar.copy(
                    out=buR[64 : 64 + M, blk0 : blk0 + 4, 1, :],
                    in_=ppv[64 : 64 + M],
                )
                yield

        def attn(c):
            # dot / dotn / transpose / exp / cs; dots in 8-batch groups
            dT = ps_dT.tile([128, 8, J + 1], bf16, tag="dT", space="PSUM")
            dn8 = [None, None]
            for h in range(2):
                b0 = c * CB + 8 * h
                dps = ps_dot.tile([J, 8, 64], f32, tag="dot", space="PSUM")
                for k in range(8):
                    b = b0 + k
                    cb = (b // CB) * CPW + (b % CB) * M
                    nc.tensor.matmul(
                        out=dps[:, k, :], lhsT=tpT[:, b * J : (b + 1) * J],
                        rhs=cpT[:, cb : cb + 64], start=True, stop=True,
                    )
                dn = trans.tile([J, 8, 64], bf16, tag="dn")
                nc.vector.tensor_tensor(
                    out=dn[:], in0=dps[:],
                    in1=ntinv[:J, b0 : b0 + 8].unsqueeze(2).broadcast_to((J, 8, 64)),
                    op=MUL,
                )
                dn8[h] = dn
                yield
            for q in range(4):
                b0 = c * CB + 4 * q
                dn = dn8[q // 2]
                for k in range(4):
                    b = b0 + k
                    po = 64 * (b % 2)
                    nc.tensor.transpose(
                        out=dT[po : po + 64, 2 * q + k // 2, :J],
                        in_=dn[:, (q % 2) * 4 + k, :], identity=eye_t[:J, :J],
                    )
                for k in range(2):
                    pr = b0 // 2 + k
                    nc.scalar.activation(
                        out=attnT[:, pr, :], in_=dT[:, 2 * q + k, :J],
                        func=AF.Exp, bias=maskP_t[:, pr : pr + 1],
                        scale=ncinvP[:, pr : pr + 1],
                    )
                for k in range(4):
                    b = b0 + k
                    nc.tensor.matmul(
                        out=small_ps[:J, 128 + b : 129 + b],
                        lhsT=attnT[0 : 64 + M, b // 2, :],
                        rhs=onesEO_t[0 : 64 + M, b % 2 : b % 2 + 1],
                        start=True, stop=True,
                    )
                nc.vector.reciprocal(
                    out=rsinv[:J, b0 : b0 + 4],
                    in_=small_ps[:J, 128 + b0 : 128 + b0 + 4],
                )
                ops = ps_o.tile([J, 4, E], f32, tag="o", space="PSUM")
                for k in range(4):
                    b = b0 + k
                    nc.tensor.matmul(
                        out=ops[:, k, :],
                        lhsT=attnT[0 : 64 + M, b // 2, :],
                        rhs=buR[0 : 64 + M, b // 2, b % 2, :],
                        start=True, stop=True,
                    )
                nc.vector.tensor_tensor(
                    out=o_all[:, b0 : b0 + 4, :], in0=ops[:],
                    in1=rsinv[:J, b0 : b0 + 4].unsqueeze(2).broadcast_to((J, 4, E)),
                    op=MUL,
                )
                if has_rb:
                    nc.gpsimd.tensor_tensor(
                        out=o_all[:, b0 : b0 + 4, :], in0=o_all[:, b0 : b0 + 4, :],
                        in1=rb_t[:J, :].unsqueeze(1).broadcast_to((J, 4, E)),
                        op=ADD,
                    )
                nc.sync.dma_start(
                    out=out[:, b0 : b0 + 4, :],
                    in_=o_all[:, b0 : b0 + 4, :],
                )
                yield

        def run(gen):
            for _ in gen:
                pass

        run(proj(0))
        run(proj(1))
        _interleave(proj(2), attn(0))
        _interleave(proj(3), attn(1))
        _interleave(attn(2), attn(3))

    orig = bacc_mod.get_activation_tables
    bacc_mod.get_activation_tables = _patched_tables(orig)
    try:
        nc.compile()
    finally:
        bacc_mod.get_activation_tables = orig
    return nc


def _get_program(has_rb=False):
    key = ("nc", bool(has_rb))
    if key not in _CACHE:
        _CACHE[key] = _build_program(has_rb)
    return _CACHE[key]


def _pack_idx(flat, n):
    """Pack a flat idx list (padded with 0 to n, n % 128 == 0) into the
    [128, n//16] int16 tile layout: idx i at [i % 16, i // 16], replicated
    8x down the partitions."""
    a = np.zeros(n, np.int16)
    a[: len(flat)] = flat
    t = a.reshape(n // 16, 16).T
    return np.tile(t, (8, 1))


def _prep_inputs(batch_titems, batch_citems, batch_pad_ids, t_emb, c_emb,
                 Ac_w, Ac_b, At_w, At_b, Bc_w, Bc_b, R_w, R_b):
    bf = ml_dtypes.bfloat16
    t_emb = np.asarray(t_emb, np.float32)
    c_emb = np.asarray(c_emb, np.float32)
    tit = np.asarray(batch_titems).astype(np.int64)
    cit = np.asarray(batch_citems).astype(np.int64)
    pad = np.asarray(batch_pad_ids).astype(np.int64)

    mask = np.zeros((B, M), np.float32)
    mask[pad[0], pad[1]] = NEG

    atwT = np.asarray(At_w, np.float32).T.astype(bf)          # [128, 60]
    acwT = np.asarray(Ac_w, np.float32).T.astype(bf)
    w2 = np.asarray(R_w, np.float32) @ np.asarray(Bc_w, np.float32)
    w2T = w2.T.astype(bf)                                     # [128, 128]
    eyeb = np.eye(128, dtype=bf)
    onesEO_np = np.zeros((128, 2), bf)
    onesEO_np[0:M, 0] = 1
    onesEO_np[64 : 64 + M, 1] = 1
    atb = np.asarray(At_b, np.float32).reshape(D, 1)
    acb = np.asarray(Ac_b, np.float32).reshape(D, 1)
    rb = (np.asarray(R_b, np.float32)
          + np.asarray(R_w, np.float32) @ np.asarray(Bc_b, np.float32))
    has_rb = bool(np.any(rb != 0))
    rbeff = rb.reshape(1, E).astype(np.float32)

    in_maps = []
    for core in range(NCORES):
        s = core * BLOC
        tslice = tit[s : s + BLOC]          # [64, 101]
        cslice = cit[s : s + BLOC]          # [64, 50]
        tvals, tinv = np.unique(tslice, return_inverse=True)
        cvals, cinv = np.unique(cslice, return_inverse=True)
        tinv = tinv.reshape(BLOC, J).astype(np.int16)
        cinv = cinv.reshape(BLOC, M).astype(np.int16)

        tcomp = np.zeros((TN, E), bf)
        tcomp[: len(tvals)] = t_emb[tvals].astype(bf)
        ccomp = np.zeros((CN, E), bf)
        ccomp[: len(cvals)] = c_emb[cvals].astype(bf)

        cols = []
        for c in range(NCHUNK):
            h0 = c * CB
            cols.append(_pack_idx(tinv[h0 : h0 + 8].reshape(-1), NIV))
            cols.append(_pack_idx(tinv[h0 + 8 : h0 + 16].reshape(-1), NIV))
            cols.append(_pack_idx(cinv[h0 : h0 + CB].reshape(-1), NIC))
        gidx = np.concatenate(cols, axis=1)   # [128, NCHUNK*(2*NIV+NIC)//16]

        mP = np.full((128, BLOC // 2), NEG, np.float32)
        mc = mask[s : s + BLOC]               # [64, 50]
        for p in range(BLOC // 2):
            mP[0:M, p] = mc[2 * p]
            mP[64 : 64 + M, p] = mc[2 * p + 1]

        in_maps.append(
            {
                "tcomp": tcomp,
                "ccomp": ccomp,
                "gidx": gidx,
                "atwT": atwT,
                "acwT": acwT,
                "w2T": w2T,
                "eyeb": eyeb,
                "atb": atb,
                "acb": acb,
                "maskP": mP,
                "onesEO": onesEO_np,
                "rbeff": rbeff,
            }
        )
    return in_maps, has_rb


def run_sharded(in_maps, has_rb, **kwargs):
    from concourse.bass_utils import run_bass_kernel_spmd

    nc = _get_program(has_rb)
    res = run_bass_kernel_spmd(nc, in_maps, core_ids=list(range(NCORES)), **kwargs)
    outs = [np.ascontiguousarray(res.results[c]["out"].transpose(1, 0, 2))
            for c in range(NCORES)]
    full = np.concatenate(outs, axis=0)
    return full, res


def kernel(**inputs):
    in_maps, has_rb = _prep_inputs(**inputs)
    full, _ = run_sharded(in_maps, has_rb)
    return full.astype(np.float32)
